# revision 2
# baseline (speedup 1.0000x reference)
"""Trainium2 Bass kernel for nn_GapDecoder.

Computes gaps[i,j] = proj[i] + proj[j] + b2 where
proj = relu(x @ W1 + b1) @ w2, x: [8192, 512] f32.

Strategy (8 NeuronCores, row-sharded):
  - Host passes each core its transposed x shard xT [512, 1024] so the PE
    can contract over D without an on-chip transpose.
  - Each core computes proj for its 1024 rows (PE matmuls + fused
    relu/bias on ACT), AllGathers the 4KB proj shard across cores,
    broadcasts the full 8192-float proj row across 128 partitions with a
    rank-1 PE matmul, then emits its [1024, 8192] output tile as 8
    chunks: DVE tensor_scalar add of the per-partition local proj,
    followed by a 4MB DMA store. The 32MB/core output write is the
    bottleneck (memory regime).
"""

import sys

sys.path.insert(0, "/opt/trn_rl_repo")

import numpy as np

N, D, H = 8192, 512, 32
NCORES = 8
LOCAL = N // NCORES  # rows per core
KCH = D // 128  # contraction chunks
STRIPE = 512  # local rows handled per PE stripe
NSTRIP = LOCAL // STRIPE
RCH = LOCAL // 128  # output row chunks per core

_state = {}

# Set by run for test harnesses that want profile info (see test.py).
LAST_RESULTS = None


def _build():
    from concourse import bacc, tile, mybir

    f32 = mybir.dt.float32
    nc = bacc.Bacc(
        "TRN2", target_bir_lowering=False, debug=False, num_devices=NCORES
    )

    xT_d = nc.dram_tensor("xT", [D, LOCAL], f32, kind="ExternalInput")
    w1_d = nc.dram_tensor("W1", [D, H], f32, kind="ExternalInput")
    b1_d = nc.dram_tensor("b1c", [H, 1], f32, kind="ExternalInput")
    w2_d = nc.dram_tensor("w2c", [H, 1], f32, kind="ExternalInput")
    b2_d = nc.dram_tensor("b2c", [1, 1], f32, kind="ExternalInput")
    out_d = nc.dram_tensor("out", [LOCAL, N], f32, kind="ExternalOutput")

    with tile.TileContext(nc) as tc:
        with (
            tc.tile_pool(name="const", bufs=1) as cpool,
            tc.tile_pool(name="work", bufs=2) as wpool,
            tc.tile_pool(name="big", bufs=3) as bigpool,
            tc.tile_pool(name="psum", bufs=2, space="PSUM") as pspool,
            tc.tile_pool(name="psbc", bufs=2, space="PSUM") as psbc,
            tc.tile_pool(name="dram", bufs=1, space="DRAM") as dram,
        ):
            # ---- constants ----
            w1_sb = cpool.tile([128, KCH, H], f32)
            nc.sync.dma_start(
                w1_sb[:], w1_d.ap().rearrange("(k p) h -> p k h", p=128)
            )
            b1_sb = cpool.tile([H, 1], f32)
            nc.sync.dma_start(b1_sb[:], b1_d.ap())
            w2_sb = cpool.tile([H, 1], f32)
            nc.sync.dma_start(w2_sb[:], w2_d.ap())
            b2_sb = cpool.tile([1, 1], f32)
            nc.sync.dma_start(b2_sb[:], b2_d.ap())
            ones_sb = cpool.tile([1, 128], f32)
            nc.vector.memset(ones_sb[:], 1.0)

            # ---- local proj: [1, LOCAL] row and [128, RCH] per-partition ----
            projloc = cpool.tile([1, LOCAL], f32)
            projcol = cpool.tile([128, RCH], f32)

            for s in range(NSTRIP):
                xk = wpool.tile([128, KCH, STRIPE], f32, tag="xk")
                nc.sync.dma_start(
                    xk[:],
                    xT_d.ap()[:, s * STRIPE : (s + 1) * STRIPE].rearrange(
                        "(k p) j -> p k j", p=128
                    ),
                )
                seqT_ps = pspool.tile([H, STRIPE], f32, tag="seqT")
                for k in range(KCH):
                    nc.tensor.matmul(
                        seqT_ps[:],
                        w1_sb[:, k, :],
                        xk[:, k, :],
                        start=(k == 0),
                        stop=(k == KCH - 1),
                    )
                seqT_sb = wpool.tile([H, STRIPE], f32, tag="seqT_sb")
                nc.scalar.activation(
                    seqT_sb[:],
                    seqT_ps[:],
                    mybir.ActivationFunctionType.Relu,
                    bias=b1_sb[:],
                    scale=1.0,
                )
                pr_ps = pspool.tile([1, STRIPE], f32, tag="pr")
                nc.tensor.matmul(pr_ps[:], w2_sb[:], seqT_sb[:])
                nc.vector.tensor_copy(
                    projloc[:, s * STRIPE : (s + 1) * STRIPE], pr_ps[:]
                )
                for c in range(STRIPE // 128):
                    pc_ps = pspool.tile([128, 1], f32, tag="pc")
                    nc.tensor.matmul(
                        pc_ps[:],
                        seqT_sb[:, c * 128 : (c + 1) * 128],
                        w2_sb[:],
                    )
                    col = s * (STRIPE // 128) + c
                    nc.vector.tensor_copy(projcol[:, col : col + 1], pc_ps[:])

            # ---- all-gather proj across the 8 cores ----
            cc_in = dram.tile([LOCAL], f32)
            cc_out = dram.tile([N], f32, addr_space="Shared")
            nc.sync.dma_start(cc_in[:], projloc[:])
            nc.gpsimd.collective_compute(
                "AllGather",
                mybir.AluOpType.bypass,
                replica_groups=[list(range(NCORES))],
                ins=[cc_in[:]],
                outs=[cc_out[:]],
            )
            g_sb = cpool.tile([1, N], f32)
            nc.sync.dma_start(g_sb[:], cc_out[:])
            # fold the output bias in once
            nc.vector.tensor_scalar_add(g_sb[:], g_sb[:], b2_sb[:])

            # ---- broadcast proj row across all 128 partitions ----
            bcast = cpool.tile([128, N], f32)
            for t in range(N // 512):
                bc_ps = psbc.tile([128, 512], f32, tag="bc")
                nc.tensor.matmul(
                    bc_ps[:], ones_sb[:], g_sb[:, t * 512 : (t + 1) * 512]
                )
                nc.vector.tensor_copy(bcast[:, t * 512 : (t + 1) * 512], bc_ps[:])

            # ---- outer sum: 8 row chunks of [128, 8192] ----
            for r in range(RCH):
                ot = bigpool.tile([128, N], f32, tag="ot")
                nc.vector.tensor_scalar_add(ot[:], bcast[:], projcol[:, r : r + 1])
                half = N // 2
                nc.sync.dma_start(
                    out_d.ap()[r * 128 : (r + 1) * 128, 0:half], ot[:, 0:half]
                )
                nc.sync.dma_start(
                    out_d.ap()[r * 128 : (r + 1) * 128, half:N], ot[:, half:N]
                )

    nc.compile()
    return nc


def kernel(gathered_sequences, W1, b1, w2, b2):
    global LAST_RESULTS
    from concourse import bass_utils

    if "nc" not in _state:
        _state["nc"] = _build()
    nc = _state["nc"]

    x = np.ascontiguousarray(gathered_sequences, dtype=np.float32)
    xT = np.ascontiguousarray(x.T)  # [D, N]
    W1 = np.ascontiguousarray(W1, dtype=np.float32)
    b1c = np.ascontiguousarray(np.reshape(b1, (H, 1)), dtype=np.float32)
    w2c = np.ascontiguousarray(np.reshape(w2, (H, 1)), dtype=np.float32)
    b2c = np.ascontiguousarray(np.reshape(b2, (1, 1)), dtype=np.float32)

    in_maps = []
    for m in range(NCORES):
        in_maps.append(
            {
                "xT": np.ascontiguousarray(
                    xT[:, m * LOCAL : (m + 1) * LOCAL]
                ),
                "W1": W1,
                "b1c": b1c,
                "w2c": w2c,
                "b2c": b2c,
            }
        )

    res = bass_utils.run_bass_kernel_spmd(nc, in_maps, core_ids=list(range(NCORES)))
    LAST_RESULTS = res
    return np.concatenate([res.results[m]["out"] for m in range(NCORES)], axis=0)


# revision 5
# speedup vs baseline: 1.4261x; 1.4261x over previous
"""Trainium2 Bass kernel for nn_GapDecoder.

Computes gaps[i,j] = proj[i] + proj[j] + b2 where
proj = relu(x @ W1 + b1) @ w2, x: [8192, 512] f32.

Strategy (8 NeuronCores, block-partitioned, collective-free):
  The [8192, 8192] output is an 8x8 grid of [1024, 1024] blocks. Core m
  handles chunk set Lm = {m, m+1, m+2, m+4} (mod 8) and emits the 8
  blocks given by the uniform local pattern
      {(0,0),(0,1),(0,2),(0,3),(1,3),(1,0),(3,1),(3,2)}
  over Lm. One cell per difference delta = Lm[q]-Lm[p] (mod 8) makes the
  union over cores an exact partition of all 64 blocks. Each core reads
  just its 4 x-chunks (8MB, transposed on host so the PE contracts over
  D directly), computes proj for those 4096 rows, broadcasts the
  column-direction proj across partitions with rank-1 PE matmuls, and
  writes each block as 8 chunks of [128, 1024]: DVE tensor_scalar add of
  the per-partition row proj, then a DMA store. 40MB of HBM traffic per
  core (vs 48MB row-sharded) and no cross-core dependency, so staggered
  core starts don't serialize anything.
"""

import sys

sys.path.insert(0, "/opt/trn_rl_repo")

import numpy as np

N, D, H = 8192, 512, 32
NCORES = 8
CHUNK = 1024  # block edge / proj chunk
NLOC = 4  # chunks per core
LROWS = NLOC * CHUNK  # local rows per core
STRIPE = 512  # rows per PE stripe
NSTRIP = LROWS // STRIPE
KCH = D // 128

# local chunk offsets and the block pattern (see module docstring)
LOCAL_OFFS = (0, 1, 2, 4)
PATTERN = ((0, 0), (0, 1), (0, 2), (0, 3), (1, 3), (1, 0), (3, 1), (3, 2))

_state = {}

# Set by run for test harnesses that want profile info (see test.py).
LAST_RESULTS = None


def _build():
    from concourse import bacc, tile, mybir

    f32 = mybir.dt.float32
    nc = bacc.Bacc(
        "TRN2", target_bir_lowering=False, debug=False, num_devices=NCORES
    )

    xT_d = nc.dram_tensor("xT4", [D, LROWS], f32, kind="ExternalInput")
    w1_d = nc.dram_tensor("W1", [D, H], f32, kind="ExternalInput")
    b1_d = nc.dram_tensor("b1c", [H, 1], f32, kind="ExternalInput")
    w2_d = nc.dram_tensor("w2c", [H, 1], f32, kind="ExternalInput")
    b2_d = nc.dram_tensor("b2c", [1, 1], f32, kind="ExternalInput")
    # 8 blocks of [CHUNK, CHUNK], stacked along rows
    out_d = nc.dram_tensor("out", [8 * CHUNK, CHUNK], f32, kind="ExternalOutput")

    with tile.TileContext(nc) as tc:
        with (
            tc.tile_pool(name="const", bufs=1) as cpool,
            tc.tile_pool(name="work", bufs=2) as wpool,
            tc.tile_pool(name="big", bufs=6) as bigpool,
            tc.tile_pool(name="psum", bufs=2, space="PSUM") as pspool,
            tc.tile_pool(name="psbc", bufs=2, space="PSUM") as psbc,
        ):
            # ---- constants ----
            w1_sb = cpool.tile([128, KCH, H], f32)
            nc.sync.dma_start(
                w1_sb[:], w1_d.ap().rearrange("(k p) h -> p k h", p=128)
            )
            b1_sb = cpool.tile([H, 1], f32)
            nc.sync.dma_start(b1_sb[:], b1_d.ap())
            w2_sb = cpool.tile([H, 1], f32)
            nc.sync.dma_start(w2_sb[:], w2_d.ap())
            b2_sb = cpool.tile([1, 1], f32)
            nc.sync.dma_start(b2_sb[:], b2_d.ap())
            ones_sb = cpool.tile([1, 128], f32)
            nc.vector.memset(ones_sb[:], 1.0)

            # per-local-chunk proj row pieces ([1, CHUNK]) and per-partition
            # scalars ([128, CHUNK//128] per local chunk)
            projrow = [
                cpool.tile([1, CHUNK], f32, name=f"projrow{i}") for i in range(NLOC)
            ]
            projcol = [
                cpool.tile([128, CHUNK // 128], f32, name=f"projcol{i}")
                for i in range(NLOC)
            ]

            # ---- proj for the 4 local chunks, stripe by stripe ----
            for s in range(NSTRIP):
                loc = s // (CHUNK // STRIPE)  # local chunk of this stripe
                half = s % (CHUNK // STRIPE)  # position within the chunk
                xk = wpool.tile([128, KCH, STRIPE], f32, tag="xk")
                nc.sync.dma_start(
                    xk[:],
                    xT_d.ap()[:, s * STRIPE : (s + 1) * STRIPE].rearrange(
                        "(k p) j -> p k j", p=128
                    ),
                )
                seqT_ps = pspool.tile([H, STRIPE], f32, tag="seqT")
                for k in range(KCH):
                    nc.tensor.matmul(
                        seqT_ps[:],
                        w1_sb[:, k, :],
                        xk[:, k, :],
                        start=(k == 0),
                        stop=(k == KCH - 1),
                    )
                seqT_sb = wpool.tile([H, STRIPE], f32, tag="seqT_sb")
                nc.scalar.activation(
                    seqT_sb[:],
                    seqT_ps[:],
                    mybir.ActivationFunctionType.Relu,
                    bias=b1_sb[:],
                    scale=1.0,
                )
                pr_ps = pspool.tile([1, STRIPE], f32, tag="pr")
                nc.tensor.matmul(pr_ps[:], w2_sb[:], seqT_sb[:])
                # fold b2 into the column-direction proj once
                nc.vector.tensor_scalar_add(
                    projrow[loc][:, half * STRIPE : (half + 1) * STRIPE],
                    pr_ps[:],
                    b2_sb[:],
                )
                for c in range(STRIPE // 128):
                    pc_ps = pspool.tile([128, 1], f32, tag="pc")
                    nc.tensor.matmul(
                        pc_ps[:],
                        seqT_sb[:, c * 128 : (c + 1) * 128],
                        w2_sb[:],
                    )
                    col = half * (STRIPE // 128) + c
                    nc.vector.tensor_copy(projcol[loc][:, col : col + 1], pc_ps[:])

            # ---- broadcast each proj chunk across all 128 partitions ----
            bcol = [
                cpool.tile([128, CHUNK], f32, name=f"bcol{i}") for i in range(NLOC)
            ]
            for q in range(NLOC):
                for t in range(CHUNK // 512):
                    bc_ps = psbc.tile([128, 512], f32, tag="bc")
                    nc.tensor.matmul(
                        bc_ps[:],
                        ones_sb[:],
                        projrow[q][:, t * 512 : (t + 1) * 512],
                    )
                    nc.vector.tensor_copy(
                        bcol[q][:, t * 512 : (t + 1) * 512], bc_ps[:]
                    )

            # ---- emit the 8 blocks (ordered by data readiness) ----
            order = sorted(range(8), key=lambda i: max(PATTERN[i]))
            for k in order:
                p, q = PATTERN[k]
                for g in range(CHUNK // 128):
                    ot = bigpool.tile([128, CHUNK], f32, tag="ot")
                    nc.vector.tensor_scalar_add(
                        ot[:], bcol[q][:], projcol[p][:, g : g + 1]
                    )
                    r0 = k * CHUNK + g * 128
                    nc.sync.dma_start(out_d.ap()[r0 : r0 + 128, :], ot[:])

    nc.compile()
    return nc


def kernel(gathered_sequences, W1, b1, w2, b2):
    global LAST_RESULTS
    from concourse import bass_utils

    if "nc" not in _state:
        _state["nc"] = _build()
    nc = _state["nc"]

    x = np.ascontiguousarray(gathered_sequences, dtype=np.float32)
    xT = np.ascontiguousarray(x.T)  # [D, N]
    W1 = np.ascontiguousarray(W1, dtype=np.float32)
    b1c = np.ascontiguousarray(np.reshape(b1, (H, 1)), dtype=np.float32)
    w2c = np.ascontiguousarray(np.reshape(w2, (H, 1)), dtype=np.float32)
    b2c = np.ascontiguousarray(np.reshape(b2, (1, 1)), dtype=np.float32)

    in_maps = []
    for m in range(NCORES):
        locs = [(m + a) % NCORES for a in LOCAL_OFFS]
        xT4 = np.concatenate(
            [xT[:, L * CHUNK : (L + 1) * CHUNK] for L in locs], axis=1
        )
        in_maps.append(
            {
                "xT4": np.ascontiguousarray(xT4),
                "W1": W1,
                "b1c": b1c,
                "w2c": w2c,
                "b2c": b2c,
            }
        )

    res = bass_utils.run_bass_kernel_spmd(nc, in_maps, core_ids=list(range(NCORES)))
    LAST_RESULTS = res

    out = np.empty((N, N), dtype=np.float32)
    for m in range(NCORES):
        locs = [(m + a) % NCORES for a in LOCAL_OFFS]
        blocks = res.results[m]["out"]
        for k, (p, q) in enumerate(PATTERN):
            gr, gc = locs[p], locs[q]
            out[gr * CHUNK : (gr + 1) * CHUNK, gc * CHUNK : (gc + 1) * CHUNK] = (
                blocks[k * CHUNK : (k + 1) * CHUNK, :]
            )
    return out


# revision 9
# speedup vs baseline: 1.4309x; 1.0033x over previous
"""Trainium2 Bass kernel for nn_GapDecoder.

Computes gaps[i,j] = proj[i] + proj[j] + b2 where
proj = relu(x @ W1 + b1) @ w2, x: [8192, 512] f32.

Strategy (8 NeuronCores, block-partitioned, collective-free):
  The [8192, 8192] output is an 8x8 grid of [1024, 1024] blocks. Core m
  handles chunk set Lm = {m, m+1, m+2, m+4} (mod 8) and emits the 8
  blocks given by the uniform local pattern
      {(0,0),(0,1),(0,2),(0,3),(1,3),(1,0),(3,1),(3,2)}
  over Lm. One cell per difference delta = Lm[q]-Lm[p] (mod 8) makes the
  union over cores an exact partition of all 64 blocks. Each core reads
  just its 4 x-chunks (8MB, transposed on host so the PE contracts over
  D directly), computes proj for those 4096 rows, broadcasts the
  column-direction proj across partitions with rank-1 PE matmuls, and
  writes each block as 8 chunks of [128, 1024]: DVE tensor_scalar add of
  the per-partition row proj, then a DMA store. 40MB of HBM traffic per
  core (vs 48MB row-sharded) and no cross-core dependency, so staggered
  core starts don't serialize anything.
"""

import sys

sys.path.insert(0, "/opt/trn_rl_repo")

import numpy as np

N, D, H = 8192, 512, 32
NCORES = 8
CHUNK = 1024  # block edge / proj chunk
NLOC = 4  # chunks per core
LROWS = NLOC * CHUNK  # local rows per core
STRIPE = 512  # rows per PE stripe
NSTRIP = LROWS // STRIPE
KCH = D // 128

# local chunk offsets and the block pattern (see module docstring)
LOCAL_OFFS = (0, 1, 2, 4)
PATTERN = ((0, 0), (0, 1), (0, 2), (0, 3), (1, 3), (1, 0), (3, 1), (3, 2))

_state = {}

# Set by run for test harnesses that want profile info (see test.py).
LAST_RESULTS = None


def _build():
    from concourse import bacc, tile, mybir

    f32 = mybir.dt.float32
    nc = bacc.Bacc(
        "TRN2", target_bir_lowering=False, debug=False, num_devices=NCORES
    )

    xT_d = nc.dram_tensor("xT4", [D, LROWS], f32, kind="ExternalInput")
    w1_d = nc.dram_tensor("W1", [D, H], f32, kind="ExternalInput")
    b1_d = nc.dram_tensor("b1c", [H, 1], f32, kind="ExternalInput")
    w2_d = nc.dram_tensor("w2c", [H, 1], f32, kind="ExternalInput")
    b2_d = nc.dram_tensor("b2c", [1, 1], f32, kind="ExternalInput")
    # 8 blocks of [CHUNK, CHUNK], stacked along rows
    out_d = nc.dram_tensor("out", [8 * CHUNK, CHUNK], f32, kind="ExternalOutput")

    with tile.TileContext(nc) as tc:
        with (
            tc.tile_pool(name="const", bufs=1) as cpool,
            tc.tile_pool(name="xkp", bufs=5) as xkpool,
            tc.tile_pool(name="work", bufs=2) as wpool,
            tc.tile_pool(name="big", bufs=8) as bigpool,
            tc.tile_pool(name="psum", bufs=2, space="PSUM") as pspool,
            tc.tile_pool(name="psbc", bufs=2, space="PSUM") as psbc,
        ):
            # ---- constants ----
            w1_sb = cpool.tile([128, KCH, H], f32)
            nc.sync.dma_start(
                w1_sb[:], w1_d.ap().rearrange("(k p) h -> p k h", p=128)
            )
            b1_sb = cpool.tile([H, 1], f32)
            nc.sync.dma_start(b1_sb[:], b1_d.ap())
            w2_sb = cpool.tile([H, 1], f32)
            nc.sync.dma_start(w2_sb[:], w2_d.ap())
            b2_sb = cpool.tile([1, 1], f32)
            nc.sync.dma_start(b2_sb[:], b2_d.ap())
            ones_sb = cpool.tile([1, 128], f32)
            nc.vector.memset(ones_sb[:], 1.0)

            # per-local-chunk proj row pieces ([1, CHUNK]) and per-partition
            # scalars ([128, CHUNK//128] per local chunk)
            projrow = [
                cpool.tile([1, CHUNK], f32, name=f"projrow{i}") for i in range(NLOC)
            ]
            projcol = [
                cpool.tile([128, CHUNK // 128], f32, name=f"projcol{i}")
                for i in range(NLOC)
            ]

            # ---- proj for the 4 local chunks, stripe by stripe ----
            # chunk compute order 0,1,3,2: more blocks become writable
            # early (see the readiness-ordered emission below)
            for loc in (0, 1, 3, 2):
              for half in range(CHUNK // STRIPE):
                s = loc * (CHUNK // STRIPE) + half
                xk = xkpool.tile([128, KCH, STRIPE], f32, tag="xk")
                nc.sync.dma_start(
                    xk[:],
                    xT_d.ap()[:, s * STRIPE : (s + 1) * STRIPE].rearrange(
                        "(k p) j -> p k j", p=128
                    ),
                )
                seqT_ps = pspool.tile([H, STRIPE], f32, tag="seqT")
                for k in range(KCH):
                    nc.tensor.matmul(
                        seqT_ps[:],
                        w1_sb[:, k, :],
                        xk[:, k, :],
                        start=(k == 0),
                        stop=(k == KCH - 1),
                    )
                seqT_sb = wpool.tile([H, STRIPE], f32, tag="seqT_sb")
                nc.scalar.activation(
                    seqT_sb[:],
                    seqT_ps[:],
                    mybir.ActivationFunctionType.Relu,
                    bias=b1_sb[:],
                    scale=1.0,
                )
                pr_ps = pspool.tile([1, STRIPE], f32, tag="pr")
                nc.tensor.matmul(pr_ps[:], w2_sb[:], seqT_sb[:])
                # fold b2 into the column-direction proj once
                nc.vector.tensor_scalar_add(
                    projrow[loc][:, half * STRIPE : (half + 1) * STRIPE],
                    pr_ps[:],
                    b2_sb[:],
                )
                # local 2 never appears as a block row; skip its scalars
                for c in range(STRIPE // 128) if loc != 2 else ():
                    pc_ps = pspool.tile([128, 1], f32, tag="pc")
                    nc.tensor.matmul(
                        pc_ps[:],
                        seqT_sb[:, c * 128 : (c + 1) * 128],
                        w2_sb[:],
                    )
                    col = half * (STRIPE // 128) + c
                    nc.vector.tensor_copy(projcol[loc][:, col : col + 1], pc_ps[:])

            # ---- broadcast each proj chunk across all 128 partitions ----
            bcol = [
                cpool.tile([128, CHUNK], f32, name=f"bcol{i}") for i in range(NLOC)
            ]
            for q in range(NLOC):
                for t in range(CHUNK // 512):
                    bc_ps = psbc.tile([128, 512], f32, tag="bc")
                    nc.tensor.matmul(
                        bc_ps[:],
                        ones_sb[:],
                        projrow[q][:, t * 512 : (t + 1) * 512],
                    )
                    nc.vector.tensor_copy(
                        bcol[q][:, t * 512 : (t + 1) * 512], bc_ps[:]
                    )

            # ---- emit the 8 blocks (ordered by data readiness under the
            # 0,1,3,2 chunk compute order) ----
            ready = {0: 0, 1: 1, 3: 2, 2: 3}
            order = sorted(
                range(8), key=lambda i: max(ready[c] for c in PATTERN[i])
            )
            for k in order:
                p, q = PATTERN[k]
                for g in range(CHUNK // 128):
                    ot = bigpool.tile([128, CHUNK], f32, tag="ot")
                    nc.vector.tensor_scalar_add(
                        ot[:], bcol[q][:], projcol[p][:, g : g + 1]
                    )
                    r0 = k * CHUNK + g * 128
                    nc.sync.dma_start(out_d.ap()[r0 : r0 + 128, :], ot[:])

    nc.compile()
    return nc


def kernel(gathered_sequences, W1, b1, w2, b2):
    global LAST_RESULTS
    from concourse import bass_utils

    if "nc" not in _state:
        _state["nc"] = _build()
    nc = _state["nc"]

    x = np.ascontiguousarray(gathered_sequences, dtype=np.float32)
    xT = np.ascontiguousarray(x.T)  # [D, N]
    W1 = np.ascontiguousarray(W1, dtype=np.float32)
    b1c = np.ascontiguousarray(np.reshape(b1, (H, 1)), dtype=np.float32)
    w2c = np.ascontiguousarray(np.reshape(w2, (H, 1)), dtype=np.float32)
    b2c = np.ascontiguousarray(np.reshape(b2, (1, 1)), dtype=np.float32)

    in_maps = []
    for m in range(NCORES):
        locs = [(m + a) % NCORES for a in LOCAL_OFFS]
        xT4 = np.concatenate(
            [xT[:, L * CHUNK : (L + 1) * CHUNK] for L in locs], axis=1
        )
        in_maps.append(
            {
                "xT4": np.ascontiguousarray(xT4),
                "W1": W1,
                "b1c": b1c,
                "w2c": w2c,
                "b2c": b2c,
            }
        )

    res = bass_utils.run_bass_kernel_spmd(nc, in_maps, core_ids=list(range(NCORES)))
    LAST_RESULTS = res

    out = np.empty((N, N), dtype=np.float32)
    for m in range(NCORES):
        locs = [(m + a) % NCORES for a in LOCAL_OFFS]
        blocks = res.results[m]["out"]
        for k, (p, q) in enumerate(PATTERN):
            gr, gc = locs[p], locs[q]
            out[gr * CHUNK : (gr + 1) * CHUNK, gc * CHUNK : (gc + 1) * CHUNK] = (
                blocks[k * CHUNK : (k + 1) * CHUNK, :]
            )
    return out


# revision 12
# speedup vs baseline: 1.4431x; 1.0086x over previous
"""Trainium2 Bass kernel for nn_GapDecoder.

Computes gaps[i,j] = proj[i] + proj[j] + b2 where
proj = relu(x @ W1 + b1) @ w2, x: [8192, 512] f32.

Strategy (8 NeuronCores, block-partitioned, collective-free):
  The [8192, 8192] output is an 8x8 grid of [1024, 1024] blocks. Core m
  handles chunk set Lm = {m, m+1, m+2, m+4} (mod 8) and emits the 8
  blocks given by the uniform local pattern
      {(0,0),(0,1),(0,2),(0,3),(1,3),(1,0),(3,1),(3,2)}
  over Lm. One cell per difference delta = Lm[q]-Lm[p] (mod 8) makes the
  union over cores an exact partition of all 64 blocks. Each core reads
  just its 4 x-chunks (8MB, transposed on host so the PE contracts over
  D directly), computes proj for those 4096 rows, broadcasts the
  column-direction proj across partitions with rank-1 PE matmuls, and
  writes each block as 8 chunks of [128, 1024]: DVE tensor_scalar add of
  the per-partition row proj, then a DMA store. 40MB of HBM traffic per
  core (vs 48MB row-sharded) and no cross-core dependency, so staggered
  core starts don't serialize anything.
"""

import sys

sys.path.insert(0, "/opt/trn_rl_repo")

import numpy as np

N, D, H = 8192, 512, 32
NCORES = 8
CHUNK = 1024  # block edge / proj chunk
NLOC = 4  # chunks per core
LROWS = NLOC * CHUNK  # local rows per core
STRIPE = 512  # rows per PE stripe
NSTRIP = LROWS // STRIPE
KCH = D // 128

# local chunk offsets and the block pattern (see module docstring)
LOCAL_OFFS = (0, 1, 2, 4)
PATTERN = ((0, 0), (0, 1), (0, 2), (0, 3), (1, 3), (1, 0), (3, 1), (3, 2))

_state = {}

# Set by run for test harnesses that want profile info (see test.py).
LAST_RESULTS = None


def _build():
    from concourse import bacc, tile, mybir

    f32 = mybir.dt.float32
    nc = bacc.Bacc(
        "TRN2", target_bir_lowering=False, debug=False, num_devices=NCORES
    )

    xT_d = nc.dram_tensor("xT4", [D, LROWS], f32, kind="ExternalInput")
    w1_d = nc.dram_tensor("W1", [D, H], f32, kind="ExternalInput")
    b1_d = nc.dram_tensor("b1c", [H, 1], f32, kind="ExternalInput")
    w2_d = nc.dram_tensor("w2c", [H, 1], f32, kind="ExternalInput")
    b2_d = nc.dram_tensor("b2c", [1, 1], f32, kind="ExternalInput")
    # 8 blocks of [CHUNK, CHUNK], stacked along rows
    out_d = nc.dram_tensor("out", [8 * CHUNK, CHUNK], f32, kind="ExternalOutput")

    with tile.TileContext(nc) as tc:
        with (
            tc.tile_pool(name="const", bufs=1) as cpool,
            tc.tile_pool(name="xkp", bufs=5) as xkpool,
            tc.tile_pool(name="work", bufs=2) as wpool,
            tc.tile_pool(name="big", bufs=8) as bigpool,
            tc.tile_pool(name="psum", bufs=2, space="PSUM") as pspool,
            tc.tile_pool(name="psbc", bufs=2, space="PSUM") as psbc,
        ):
            # ---- constants ----
            w1_sb = cpool.tile([128, KCH, H], f32)
            nc.sync.dma_start(
                w1_sb[:], w1_d.ap().rearrange("(k p) h -> p k h", p=128)
            )
            b1_sb = cpool.tile([H, 1], f32)
            nc.sync.dma_start(b1_sb[:], b1_d.ap())
            w2_sb = cpool.tile([H, 1], f32)
            nc.sync.dma_start(w2_sb[:], w2_d.ap())
            b2_sb = cpool.tile([1, 1], f32)
            nc.sync.dma_start(b2_sb[:], b2_d.ap())
            ones_sb = cpool.tile([1, 128], f32)
            nc.vector.memset(ones_sb[:], 1.0)

            # per-local-chunk proj row pieces ([1, CHUNK]) and per-partition
            # scalars ([128, CHUNK//128] per local chunk)
            projrow = [
                cpool.tile([1, CHUNK], f32, name=f"projrow{i}") for i in range(NLOC)
            ]
            projcol = [
                cpool.tile([128, CHUNK // 128], f32, name=f"projcol{i}")
                for i in range(NLOC)
            ]
            bcol = [
                cpool.tile([128, CHUNK], f32, name=f"bcol{i}") for i in range(NLOC)
            ]

            # ---- per chunk: proj stripes, then its broadcast, then every
            # block that just became ready — so output DMAs start as soon
            # as the first chunk's proj exists and overlap later compute.
            COMPUTE_ORDER = (0, 1, 3, 2)
            ready = {loc: i for i, loc in enumerate(COMPUTE_ORDER)}
            emitted = set()

            def emit_block(k):
                p, q = PATTERN[k]
                for g in range(CHUNK // 128):
                    ot = bigpool.tile([128, CHUNK], f32, tag="ot", name="ot")
                    nc.vector.tensor_scalar_add(
                        ot[:], bcol[q][:], projcol[p][:, g : g + 1]
                    )
                    r0 = k * CHUNK + g * 128
                    nc.sync.dma_start(out_d.ap()[r0 : r0 + 128, :], ot[:])

            for loc in COMPUTE_ORDER:
              for half in range(CHUNK // STRIPE):
                s = loc * (CHUNK // STRIPE) + half
                xk = xkpool.tile([128, KCH, STRIPE], f32, tag="xk")
                nc.sync.dma_start(
                    xk[:],
                    xT_d.ap()[:, s * STRIPE : (s + 1) * STRIPE].rearrange(
                        "(k p) j -> p k j", p=128
                    ),
                )
                seqT_ps = pspool.tile([H, STRIPE], f32, tag="seqT")
                for k in range(KCH):
                    nc.tensor.matmul(
                        seqT_ps[:],
                        w1_sb[:, k, :],
                        xk[:, k, :],
                        start=(k == 0),
                        stop=(k == KCH - 1),
                    )
                seqT_sb = wpool.tile([H, STRIPE], f32, tag="seqT_sb")
                nc.scalar.activation(
                    seqT_sb[:],
                    seqT_ps[:],
                    mybir.ActivationFunctionType.Relu,
                    bias=b1_sb[:],
                    scale=1.0,
                )
                pr_ps = pspool.tile([1, STRIPE], f32, tag="pr")
                nc.tensor.matmul(pr_ps[:], w2_sb[:], seqT_sb[:])
                # fold b2 into the column-direction proj once
                nc.vector.tensor_scalar_add(
                    projrow[loc][:, half * STRIPE : (half + 1) * STRIPE],
                    pr_ps[:],
                    b2_sb[:],
                )
                # local 2 never appears as a block row; skip its scalars
                for c in range(STRIPE // 128) if loc != 2 else ():
                    pc_ps = pspool.tile([128, 1], f32, tag="pc")
                    nc.tensor.matmul(
                        pc_ps[:],
                        seqT_sb[:, c * 128 : (c + 1) * 128],
                        w2_sb[:],
                    )
                    col = half * (STRIPE // 128) + c
                    nc.vector.tensor_copy(projcol[loc][:, col : col + 1], pc_ps[:])

              # broadcast this chunk's proj across all 128 partitions
              for t in range(CHUNK // 512):
                  bc_ps = psbc.tile([128, 512], f32, tag="bc")
                  nc.tensor.matmul(
                      bc_ps[:],
                      ones_sb[:],
                      projrow[loc][:, t * 512 : (t + 1) * 512],
                  )
                  nc.vector.tensor_copy(
                      bcol[loc][:, t * 512 : (t + 1) * 512], bc_ps[:]
                  )

              # emit every block whose chunks are now all computed
              for k in range(8):
                  p, q = PATTERN[k]
                  if k not in emitted and max(ready[p], ready[q]) <= ready[loc]:
                      emitted.add(k)
                      emit_block(k)

    nc.compile()
    return nc


def kernel(gathered_sequences, W1, b1, w2, b2):
    global LAST_RESULTS
    from concourse import bass_utils

    if "nc" not in _state:
        _state["nc"] = _build()
    nc = _state["nc"]

    x = np.ascontiguousarray(gathered_sequences, dtype=np.float32)
    xT = np.ascontiguousarray(x.T)  # [D, N]
    W1 = np.ascontiguousarray(W1, dtype=np.float32)
    b1c = np.ascontiguousarray(np.reshape(b1, (H, 1)), dtype=np.float32)
    w2c = np.ascontiguousarray(np.reshape(w2, (H, 1)), dtype=np.float32)
    b2c = np.ascontiguousarray(np.reshape(b2, (1, 1)), dtype=np.float32)

    in_maps = []
    for m in range(NCORES):
        locs = [(m + a) % NCORES for a in LOCAL_OFFS]
        xT4 = np.concatenate(
            [xT[:, L * CHUNK : (L + 1) * CHUNK] for L in locs], axis=1
        )
        in_maps.append(
            {
                "xT4": np.ascontiguousarray(xT4),
                "W1": W1,
                "b1c": b1c,
                "w2c": w2c,
                "b2c": b2c,
            }
        )

    res = bass_utils.run_bass_kernel_spmd(nc, in_maps, core_ids=list(range(NCORES)))
    LAST_RESULTS = res

    out = np.empty((N, N), dtype=np.float32)
    for m in range(NCORES):
        locs = [(m + a) % NCORES for a in LOCAL_OFFS]
        blocks = res.results[m]["out"]
        for k, (p, q) in enumerate(PATTERN):
            gr, gc = locs[p], locs[q]
            out[gr * CHUNK : (gr + 1) * CHUNK, gc * CHUNK : (gc + 1) * CHUNK] = (
                blocks[k * CHUNK : (k + 1) * CHUNK, :]
            )
    return out
